# revision 57
# baseline (speedup 1.0000x reference)
"""Causal multi-head attention on 8 Trainium2 NeuronCores.

Problem: B=2, NH=16, T=2048, D=64 fp32.
Sharding: the 32 (batch, head) pairs are split 4-per-core; each core runs its
heads' full causal attention independently (no collectives).

Per-core kernel design (per head):
  - Scores in the log2 domain: host prescales Q^T by log2e/8, so S^T chunks
    [s=128 partitions, q free] = K_blk @ Q^T (PE, float32r) hold u with
    softmax numerators exp2(u). Causality at 128-row granularity.
  - The q axis is processed in three passes of 896/896/256 columns (7/7/2
    q-blocks) so each pass's O accumulator fits ONE PSUM bank, leaving six
    banks for three 1024-wide S^T tiles (lookahead 3 keeps exp off the PE
    critical path).
  - exp2 is split across BOTH ACT and DVE (a single-engine softmax was the
    bottleneck), alternating chunks so neither engine gates the pipeline:
      * ACT chunks: activation(Exp, scale=ln2) -> bf16 P^T. Diagonal
        masking for ACT chunks is a DVE tensor_add of a -1e9 triangle onto
        PSUM beforehand (Pool/GPSIMD cannot access PSUM).
      * DVE chunks: one-instruction Schraudolph exp2 in bf16: bits16 =
        int16(u*2^7 + B) written through an int16 bitcast; the bit pattern
        IS bf16(2^u) (max rel err ~3% on those cells, ~7e-3 end to end).
        Diagonal chunks use scalar_tensor_tensor with the magic bias baked
        into a [128, 1152] operand: masked lanes get bias - 2^20 and
        saturate to int16 min = 0x8000 = bf16 -0.0.
  - O accumulated DIRECTLY in [q,d] orientation: per (chunk item, q-block),
    PSUM[q=128, 65] += P^T_block^T @ [V | ones] with bf16 operands (65-wide
    matmuls at 1 cycle/row: half the PE columns of the O^T formulation, no
    PSUM evacuation, no transpose epilogue). Column 64 accumulates the
    softmax denominator via the ones column. Each pass's O bank holds ONE
    accumulation group (pre-zero matmul opens it, a single stop closes it):
    interleaved per-region start/stop groups in one PSUM bank corrupt
    regions that stop while others continue (verified on hardware).
  - Per-pass epilogue: DVE reciprocal of the denominator column + one fused
    broadcast multiply PSUM->SBUF (q is the partition dim, so the scale is
    per-partition), then one DMA store per pass.

The host side only reformats layouts (transpose/pack/scale/shard in numpy);
every FLOP of the attention math runs on device.
"""

import numpy as np
import ml_dtypes

import concourse.mybir as mybir
import concourse.tile as tile
from concourse import bacc
from concourse.bass_utils import run_bass_kernel_spmd

B, NH, T, D = 2, 16, 2048, 64
HPC = 4  # heads per core
NCORES = 8
NKB = T // 128  # 16 k-blocks of 128 rows
F32 = mybir.dt.float32
F32R = mybir.dt.float32r
BF16 = mybir.dt.bfloat16
I16 = mybir.dt.int16
NEG = -1.0e9
LN2 = float(np.log(2.0))
LOG2E = float(np.log2(np.e))
# bf16 Schraudolph exp2: bitcast16(int16(u*2^7 + (127*2^7 - C16))).
C16 = 7.0
B16F = float(np.float32(127.0 * 128 - C16))
MASK_DROP = 2.0**20  # masked lanes: bias - 2^20 -> int16 saturates -> -0.0

# Pass layout: q columns [0,896), [896,1792), [1792,2048).
PASSES = [(0, 896), (896, 1792), (1792, 2048)]
# Chunk tables per pass: (group_of_kbs, engine) with engine "A"=ACT, "V"=DVE.
# Diag-leading groups put the diagonal k-block first; ACT diag chunks get a
# DVE mask add (Pool cannot touch PSUM), DVE diag chunks use the fused
# scalar_tensor_tensor bias.
PASS_CHUNKS = {
    0: [([0], "V"), ([1], "V"), ([2], "V"), ([3], "V"), ([4], "V"),
        ([5], "V"), ([6], "V")],
    1: [([0], "A"), ([1], "A"), ([2], "A"), ([3], "A"), ([4], "A"),
        ([5], "A"), ([6], "A"), ([7], "A"), ([8], "A"), ([9], "V"),
        ([10], "V"), ([11], "V"), ([12], "V"), ([13], "V")],
    2: [([14, 0, 1], "V"), ([15, 2], "V"), ([3, 4, 5], "A"), ([6, 7, 8], "A"),
        ([9, 10, 11], "A"), ([12, 13], "A")],
}
# Interleaved per-head emission order (pass, chunk index): alternates the two
# exp engines exactly (14 V / 13 A) so neither serializes, keeps pass C after
# pass A (the O accumulator pool holds two passes), and places the only two
# mask-needing ACT diag chunks (B kb7/kb8) mid-head so their DVE mask adds
# never sit on the head-boundary critical path.
ORDER = [
    (0, 0), (1, 0), (0, 1), (1, 1), (0, 2), (1, 2), (0, 3), (1, 3),
    (0, 4), (1, 4), (0, 5), (1, 5), (0, 6), (1, 6), (1, 9), (1, 7),
    (1, 10), (1, 8), (1, 11), (2, 2), (1, 12), (2, 3), (1, 13), (2, 4),
    (2, 0), (2, 5), (2, 1),
]

DEFER = 2
PTP_BUFS = 6

_cached = {}


def _build(reps=1, overrides=None):
    key = ("nc", reps, repr(overrides))
    if key in _cached:
        return _cached[key]
    global PASS_CHUNKS, DEFER, PTP_BUFS
    if overrides:
        PASS_CHUNKS = overrides.get("pass_chunks", PASS_CHUNKS)
        DEFER = overrides.get("defer", DEFER)
        PTP_BUFS = overrides.get("ptp_bufs", PTP_BUFS)
    nc = bacc.Bacc("TRN2", target_bir_lowering=False, debug=False)
    # Q^T / K^T: [64, T] (d on partitions); Q^T prescaled by log2e/8
    qt = nc.dram_tensor("qt", (HPC, D, T), F32R, kind="ExternalInput").ap()
    kt = nc.dram_tensor("kt", (HPC, D, T), F32R, kind="ExternalInput").ap()
    # V augmented with a ones column, bf16, contiguous per partition:
    # [h, p, c*65 + d] = V[h, 128*c + p, d], d=64 -> 1.0
    v = nc.dram_tensor("v", (HPC, 128, NKB * (D + 1)), BF16, kind="ExternalInput").ap()
    mask = nc.dram_tensor("mask", (128, 128), F32, kind="ExternalInput").ap()
    # Schraudolph bias plane: col j<128: B16F (s<=j) / B16F-2^20 (s>j);
    # cols 128..1151: B16F. A diag-leading chunk of width w uses [:, :w].
    mbig = nc.dram_tensor("mbig", (128, 1152), F32, kind="ExternalInput").ap()
    # out [h, p, c*64 + d] = O[h, 128*c + p, d]
    o = nc.dram_tensor("o", (HPC, 128, NKB * D), F32, kind="ExternalOutput").ap()

    with tile.TileContext(nc) as tc:
        with (
            tc.tile_pool(name="constp", bufs=1) as constp,
            tc.tile_pool(name="qkp", bufs=3) as qkp,
            tc.tile_pool(name="ptp", bufs=PTP_BUFS) as ptp,
            tc.tile_pool(name="osbp", bufs=3) as osbp,
            tc.tile_pool(name="spp", bufs=3, space="PSUM") as spp,
            tc.tile_pool(name="opp", bufs=2, space="PSUM") as opp,
        ):
            mask_sb = constp.tile([128, 128], F32)
            mbig_sb = constp.tile([128, 1152], F32)

            def body():
                _emit_body(
                    nc, tc, qt, kt, v, o, mask_sb, mbig_sb, qkp, ptp, osbp,
                    spp, opp, mask, mbig,
                )

            if reps == 1:
                body()
            else:
                with tc.For_i(0, reps, 1):
                    body()

    nc.compile()
    _cached[key] = nc
    return nc


def _chunk_table(last_head=False):
    """Per-head chunk list in interleaved ORDER.

    Each chunk: dict(g, items, w, eng, diag) with items =
    [(kb, loc, c0, c1)]: kb's piece covers q columns [c0, c1) at tile-local
    offset loc. diag = True iff items[0] starts at its own diagonal.
    """
    per_pass = {}
    for g, (p0, p1) in enumerate(PASSES):
        lst = []
        for grp, eng in PASS_CHUNKS[g]:
            items, off = [], 0
            for kb in grp:
                qs = 128 * kb
                c0, c1 = max(qs, p0), p1
                w = c1 - c0
                assert w > 0
                items.append((kb, off, c0, c1))
                off += w
            assert off <= 1024
            diag = items[0][2] == 128 * items[0][0]
            # tile-local placement: shifting a single-item chunk by 128 turns
            # a 512+128 bank split into 384+256 (avoids the sub-256 fp32r
            # penalty); multi-item groups stay packed from 0.
            if len(items) == 1 and off % 512 == 128 and off > 128:
                items = [(kb, loc + 128, c0, c1) for kb, loc, c0, c1 in items]
            lst.append(
                {"g": g, "items": items, "eng": eng, "diag": diag,
                 "lo": items[0][1], "w": off}
            )
        per_pass[g] = lst
    assert sorted(ORDER) == sorted(
        (g, i) for g in per_pass for i in range(len(per_pass[g]))
    )
    chunks = [per_pass[g][i] for g, i in ORDER]
    # The PSUM O-bank must hold a SINGLE accumulation group: interleaved
    # per-region start/stop groups within one bank corrupt regions that stop
    # while others continue (verified on hardware). Each pass's bank is
    # opened by one pre-zero matmul; bank_last[g] marks the only stop.
    bank_last = {}
    for ci, ch in enumerate(chunks):
        for kb, loc, c0, c1 in ch["items"]:
            for qb in range(c0 // 128, c1 // 128):
                bank_last[ch["g"]] = (ci, kb, qb)
    return chunks, bank_last


def _emit_body(
    nc, tc, qt, kt, v, o, mask_sb, mbig_sb, qkp, ptp, osbp, spp, opp, mask, mbig
):
    EXP = mybir.ActivationFunctionType.Exp
    tables = [_chunk_table(last_head=(h == HPC - 1)) for h in range(HPC)]
    npc = len(tables[0][0])  # chunks per head

    sb = {}  # h -> (qt_sb, kt_sb, v_sb)
    heads = {}  # h -> dict(o_sb, rec)
    opsum = {}  # (h, g) -> psum tile [128, 512]
    sch_tiles = {}  # (h, ci) -> sch tile

    # Warm the ACT exp table immediately (overlaps the first input DMAs).
    warm = osbp.tile([128, 1], F32, tag="warm")
    nc.vector.memset(warm[:], 0.0)
    nc.scalar.activation(warm[:], warm[:], EXP, scale=0.0)
    # zeros operand for the O-bank pre-zero matmuls
    zb = osbp.tile([128, 512], BF16, tag="zb")
    nc.vector.memset(zb[:], 0.0)

    def load(h, first_=False):
        qt_sb = qkp.tile([D, T], F32R, tag="qt", name=f"qt_sb{h}")
        kt_sb = qkp.tile([D, T], F32R, tag="kt", name=f"kt_sb{h}")
        v_sb = qkp.tile([128, NKB * (D + 1)], BF16, tag="v", name=f"v_sb{h}")
        if first_:
            # fine-grained leading pieces spread over FOUR queues (scalar +
            # sync + vector HWDGE rings, gpsimd SWDGE) so the whole first
            # head lands in ~4 us. The interleaved chunk order consumes
            # pass-B columns (q >= 896) from chunk 3 on, so the vector ring
            # carries the qt tail from the start.
            nc.scalar.dma_start(kt_sb[:, :128], kt[h, :, :128])
            nc.sync.dma_start(qt_sb[:, :512], qt[h, :, :512])
            nc.sync.dma_start(qt_sb[:, 512:896], qt[h, :, 512:896])
            nc.gpsimd.dma_start(qt_sb[:, 896:], qt[h, :, 896:])
            nc.gpsimd.dma_start(kt_sb[:, 128 : T // 2], kt[h, :, 128 : T // 2])
            nc.gpsimd.dma_start(v_sb[:, : 8 * 65], v[h, :, : 8 * 65])
        else:
            # coarse prefetch on the sync HWDGE ring (cheap triggers; the
            # latency slack is ~1.5 head-periods)
            nc.sync.dma_start(kt_sb[:], kt[h])
            nc.sync.dma_start(qt_sb[:], qt[h])
            nc.sync.dma_start(v_sb[:], v[h])
        sb[h] = (qt_sb, kt_sb, v_sb)

    def load_tail(h):
        qt_sb, kt_sb, v_sb = sb[h]
        nc.gpsimd.dma_start(kt_sb[:, T // 2 :], kt[h, :, T // 2 :])
        nc.gpsimd.dma_start(v_sb[:, 8 * 65 :], v[h, :, 8 * 65 :])

    def emit_S(h, ci):
        ch = tables[h][0][ci]
        qt_sb, kt_sb, _ = sb[h]
        sch = spp.tile([128, 1024], F32, tag="s", name=f"sch{h}_{ci}")

        for kb, loc, c0, c1 in ch["items"]:
            qs = 128 * kb
            w = c1 - c0
            p = 0
            while p < w:
                pl = min(w, ((loc + p) // 512 + 1) * 512 - loc)
                nc.tensor.matmul(
                    sch[:, loc + p : loc + pl],
                    lhsT=kt_sb[:, qs : qs + 128],
                    rhs=qt_sb[:, c0 + p : c0 + pl],
                    start=True,
                    stop=True,
                )
                p = pl
        if ch["diag"] and ch["eng"] == "A":
            # Pool/GPSIMD cannot touch PSUM (BIR verifier) -> DVE mask add.
            kb0, loc0, _, _ = ch["items"][0]
            nc.vector.tensor_add(
                sch[:, loc0 : loc0 + 128], sch[:, loc0 : loc0 + 128], mask_sb[:]
            )
        sch_tiles[(h, ci)] = sch

    def emit_exp(h, ci):
        ch = tables[h][0][ci]
        lo, w = ch["lo"], ch["w"]
        sch = sch_tiles.pop((h, ci))
        ptt = ptp.tile([128, 1024], BF16, tag="pt", name=f"ptt{h}_{ci}")
        if ch["eng"] == "V":
            if ch["diag"]:
                nc.vector.scalar_tensor_tensor(
                    ptt[:, lo : lo + w].bitcast(I16),
                    sch[:, lo : lo + w],
                    128.0,
                    mbig_sb[:, :w],
                    mybir.AluOpType.mult,
                    mybir.AluOpType.add,
                )
            else:
                nc.vector.tensor_scalar(
                    ptt[:, lo : lo + w].bitcast(I16),
                    sch[:, lo : lo + w],
                    128.0,
                    B16F,
                    mybir.AluOpType.mult,
                    mybir.AluOpType.add,
                )
        else:
            nc.scalar.activation(
                ptt[:, lo : lo + w], sch[:, lo : lo + w], EXP, scale=LN2
            )
        return ptt

    def emit_O(h, ci, ptt):
        ch = tables[h][0][ci]
        g = ch["g"]
        _, _, v_sb = sb[h]
        qb0_pass = PASSES[g][0] // 128
        nqb = PASSES[g][1] // 128 - qb0_pass
        if (h, g) not in opsum:
            ob = opp.tile([128, 512], F32, tag="ob", name=f"ob{h}_{g}")
            opsum[(h, g)] = ob
            # open the bank's single accumulation group, zeroing all regions
            nc.tensor.matmul(
                ob[:, : 65 * nqb],
                lhsT=zb[:, :128],
                rhs=zb[:, : 65 * nqb],
                start=True,
                stop=False,
                skip_group_check=True,
            )
        ob = opsum[(h, g)]
        for kb, loc, c0, c1 in ch["items"]:
            for qb in range(c0 // 128, c1 // 128):
                j = qb - qb0_pass
                off = loc + 128 * qb - c0
                nc.tensor.matmul(
                    ob[:, 65 * j : 65 * j + 65],
                    lhsT=ptt[:, off : off + 128],
                    rhs=v_sb[:, 65 * kb : 65 * kb + 65],
                    start=False,
                    stop=(tables[h][1][g] == (ci, kb, qb)),
                    skip_group_check=True,
                )

    def emit_epilogue(h, g):
        if h not in heads:
            heads[h] = {
                "o": osbp.tile([128, NKB * D], F32, tag="o", name=f"o_sb{h}"),
                "rec": osbp.tile([128, NKB], F32, tag="rec", name=f"rec{h}"),
            }
        o_sb, rec = heads[h]["o"], heads[h]["rec"]
        ob = opsum.pop((h, g))
        qb0 = PASSES[g][0] // 128
        nqb = PASSES[g][1] // 128 - qb0
        sums = ob[:, : nqb * 65].rearrange("p (c d) -> p c d", d=65)[:, :, 64]
        nc.vector.reciprocal(rec[:, qb0 : qb0 + nqb], sums)
        nc.vector.tensor_mul(
            o_sb[:, 64 * qb0 : 64 * (qb0 + nqb)].rearrange(
                "p (c d) -> p c d", d=64
            ),
            ob[:, : nqb * 65].rearrange("p (c d) -> p c d", d=65)[:, :, :64],
            rec[:, qb0 : qb0 + nqb].unsqueeze(2).to_broadcast((128, nqb, 64)),
        )
        nc.sync.dma_start(
            o[h, :, 64 * qb0 : 64 * (qb0 + nqb)],
            o_sb[:, 64 * qb0 : 64 * (qb0 + nqb)],
        )

    LOOKAHEAD = 2
    n = HPC * npc
    load(0, first_=True)
    # mbig is needed by chunk 0 (DVE diagonal): scalar ring (otherwise
    # idle). mask (first ACT diagonal, chunk 1) on sync behind the leading
    # qt pieces.
    nc.scalar.dma_start(mbig_sb[:], mbig[:])
    nc.sync.dma_start(mask_sb[:], mask[:])
    load(1, first_=False)
    for i in range(min(LOOKAHEAD, n)):
        emit_S(i // npc, i % npc)
    pass_count = {g: len(PASS_CHUNKS[g]) for g in PASS_CHUNKS}
    deferred = {}  # emit-at global idx -> [(h, g) epilogues]
    done_cnt = {}
    for i in range(n):
        h, ci = i // npc, i % npc
        ch = tables[h][0][ci]
        ptt = emit_exp(h, ci)
        if i == 1:
            load_tail(0)
        if i + LOOKAHEAD < n:
            j = i + LOOKAHEAD
            emit_S(j // npc, j % npc)
        emit_O(h, ci, ptt)
        for hg in deferred.pop(i, []):
            emit_epilogue(*hg)
        g = ch["g"]
        done_cnt[(h, g)] = done_cnt.get((h, g), 0) + 1
        if done_cnt[(h, g)] == pass_count[g]:
            deferred.setdefault(min(i + DEFER, n - 1), []).append((h, g))
            if g == 0 and h + 2 < HPC:
                load(h + 2)
    for i in sorted(deferred):
        for hg in deferred[i]:
            emit_epilogue(*hg)
    deferred.clear()


def _prep_in_maps(Q, K, V):
    Q = np.asarray(Q, dtype=np.float32).reshape(B * NH, T, D)
    K = np.asarray(K, dtype=np.float32).reshape(B * NH, T, D)
    V = np.asarray(V, dtype=np.float32).reshape(B * NH, T, D)

    mask = np.where(
        np.arange(128)[:, None] <= np.arange(128)[None, :], 0.0, NEG
    ).astype(np.float32)
    tri = np.where(
        np.arange(128)[:, None] <= np.arange(128)[None, :], B16F, B16F - MASK_DROP
    ).astype(np.float32)
    mbig = np.concatenate(
        [tri, np.full((128, 1024), B16F, dtype=np.float32)], axis=1
    )
    qscale = np.float32(LOG2E / np.sqrt(D))

    in_maps = []
    for c in range(NCORES):
        hs = slice(HPC * c, HPC * (c + 1))
        qt = Q[hs].transpose(0, 2, 1) * qscale  # [hpc, 64, T], log2-domain
        kt = K[hs].transpose(0, 2, 1)
        va = np.concatenate(
            [V[hs], np.ones((HPC, T, 1), dtype=np.float32)], axis=-1
        )  # [hpc, T, 65]
        va = (
            va.reshape(HPC, NKB, 128, D + 1)
            .transpose(0, 2, 1, 3)
            .reshape(HPC, 128, NKB * (D + 1))
            .astype(ml_dtypes.bfloat16)
        )
        in_maps.append(
            {
                "qt": np.ascontiguousarray(qt.astype(np.float32)),
                "kt": np.ascontiguousarray(kt),
                "v": np.ascontiguousarray(va),
                "mask": mask,
                "mbig": mbig,
            }
        )
    return in_maps


def _gather(results):
    out = np.empty((B * NH, T, D), dtype=np.float32)
    for c in range(NCORES):
        oc = results[c]["o"]  # [HPC, 128, NKB*D]
        for s in range(HPC):
            out[HPC * c + s] = (
                oc[s].reshape(128, NKB, D).transpose(1, 0, 2).reshape(T, D)
            )
    return out.reshape(B, NH, T, D)


def _run(in_maps, **kwargs):
    nc = _build()
    return run_bass_kernel_spmd(nc, in_maps, core_ids=list(range(NCORES)), **kwargs)


def kernel(Q, K, V):
    in_maps = _prep_in_maps(Q, K, V)
    res = _run(in_maps)
    return _gather(res.results)


# revision 61
# speedup vs baseline: 1.0029x; 1.0029x over previous
"""Causal multi-head attention on 8 Trainium2 NeuronCores.

Problem: B=2, NH=16, T=2048, D=64 fp32.
Sharding: the 32 (batch, head) pairs are split 4-per-core; each core runs its
heads' full causal attention independently (no collectives).

Per-core kernel design (per head):
  - Scores in the log2 domain: host prescales Q^T by log2e/8, so S^T chunks
    [s=128 partitions, q free] = K_blk @ Q^T (PE, float32r) hold u with
    softmax numerators exp2(u). Causality at 128-row granularity.
  - The q axis is processed in three passes of 896/896/256 columns (7/7/2
    q-blocks) so each pass's O accumulator fits ONE PSUM bank, leaving six
    banks for three 1024-wide S^T tiles (lookahead 3 keeps exp off the PE
    critical path).
  - exp2 is split across BOTH ACT and DVE (a single-engine softmax was the
    bottleneck), alternating chunks so neither engine gates the pipeline:
      * ACT chunks: activation(Exp, scale=ln2) -> bf16 P^T. Diagonal
        masking for ACT chunks is a DVE tensor_add of a -1e9 triangle onto
        PSUM beforehand (Pool/GPSIMD cannot access PSUM).
      * DVE chunks: one-instruction Schraudolph exp2 in bf16: bits16 =
        int16(u*2^7 + B) written through an int16 bitcast; the bit pattern
        IS bf16(2^u) (max rel err ~3% on those cells, ~7e-3 end to end).
        Diagonal chunks use scalar_tensor_tensor with the magic bias baked
        into a [128, 1152] operand: masked lanes get bias - 2^20 and
        saturate to int16 min = 0x8000 = bf16 -0.0.
  - O accumulated DIRECTLY in [q,d] orientation: per (chunk item, q-block),
    PSUM[q=128, 65] += P^T_block^T @ [V | ones] with bf16 operands (65-wide
    matmuls at 1 cycle/row: half the PE columns of the O^T formulation, no
    PSUM evacuation, no transpose epilogue). Column 64 accumulates the
    softmax denominator via the ones column. Each pass's O bank holds ONE
    accumulation group (pre-zero matmul opens it, a single stop closes it):
    interleaved per-region start/stop groups in one PSUM bank corrupt
    regions that stop while others continue (verified on hardware).
  - Per-pass epilogue: DVE reciprocal of the denominator column + one fused
    broadcast multiply PSUM->SBUF (q is the partition dim, so the scale is
    per-partition), then one DMA store per pass.

The host side only reformats layouts (transpose/pack/scale/shard in numpy);
every FLOP of the attention math runs on device.
"""

import numpy as np
import ml_dtypes

import concourse.mybir as mybir
import concourse.tile as tile
from concourse import bacc
from concourse.bass_utils import run_bass_kernel_spmd

B, NH, T, D = 2, 16, 2048, 64
HPC = 4  # heads per core
NCORES = 8
NKB = T // 128  # 16 k-blocks of 128 rows
F32 = mybir.dt.float32
F32R = mybir.dt.float32r
BF16 = mybir.dt.bfloat16
I16 = mybir.dt.int16
NEG = -1.0e9
LN2 = float(np.log(2.0))
LOG2E = float(np.log2(np.e))
# bf16 Schraudolph exp2: bitcast16(int16(u*2^7 + (127*2^7 - C16))).
C16 = 7.0
B16F = float(np.float32(127.0 * 128 - C16))
MASK_DROP = 2.0**20  # masked lanes: bias - 2^20 -> int16 saturates -> -0.0

# Pass layout: q columns [0,896), [896,1792), [1792,2048).
PASSES = [(0, 896), (896, 1792), (1792, 2048)]
# Chunk tables per pass: (group_of_kbs, engine) with engine "A"=ACT, "V"=DVE.
# Diag-leading groups put the diagonal k-block first; ACT diag chunks get a
# DVE mask add (Pool cannot touch PSUM), DVE diag chunks use the fused
# scalar_tensor_tensor bias.
PASS_CHUNKS = {
    0: [([0], "V"), ([1], "V"), ([2], "V"), ([3], "V"), ([4], "V"),
        ([5], "V"), ([6], "V")],
    1: [([0], "A"), ([1], "A"), ([2], "A"), ([3], "A"), ([4], "A"),
        ([5], "A"), ([6], "A"), ([7], "A"), ([8], "A"),
        ([9, 12], "V", (128, 768)), ([10, 13], "V", (0, 512)),
        ([11], "V")],
    2: [([14, 0, 1], "V"), ([15, 2], "V"), ([3, 4, 5], "A"), ([6, 7, 8], "A"),
        ([9, 10, 11], "A"), ([12, 13], "A")],
}
# Interleaved per-head emission order (pass, chunk index): alternates the two
# exp engines exactly (14 V / 13 A) so neither serializes, keeps pass C after
# pass A (the O accumulator pool holds two passes), and places the only two
# mask-needing ACT diag chunks (B kb7/kb8) mid-head so their DVE mask adds
# never sit on the head-boundary critical path.
ORDER = [
    (0, 0), (1, 0), (0, 1), (1, 1), (0, 2), (1, 2), (0, 3), (1, 3),
    (0, 4), (1, 4), (0, 5), (1, 5), (0, 6), (1, 6), (1, 9), (1, 7),
    (1, 10), (1, 8), (1, 11), (2, 2), (2, 0), (2, 3), (2, 1), (2, 4),
    (2, 5),
]

# multi-diagonal fused-mask bins -> bias-plane index in the mplane const
MBIN_PLANES = {(9, 12): 1, (10, 13): 2}

DEFER = 2
PTP_BUFS = 6

_cached = {}


def _build(reps=1, overrides=None):
    key = ("nc", reps, repr(overrides))
    if key in _cached:
        return _cached[key]
    global PASS_CHUNKS, DEFER, PTP_BUFS
    if overrides:
        PASS_CHUNKS = overrides.get("pass_chunks", PASS_CHUNKS)
        DEFER = overrides.get("defer", DEFER)
        PTP_BUFS = overrides.get("ptp_bufs", PTP_BUFS)
    nc = bacc.Bacc("TRN2", target_bir_lowering=False, debug=False)
    # Q^T / K^T: [64, T] (d on partitions); Q^T prescaled by log2e/8
    qt = nc.dram_tensor("qt", (HPC, D, T), F32R, kind="ExternalInput").ap()
    kt = nc.dram_tensor("kt", (HPC, D, T), F32R, kind="ExternalInput").ap()
    # V augmented with a ones column, bf16, contiguous per partition:
    # [h, p, c*65 + d] = V[h, 128*c + p, d], d=64 -> 1.0
    v = nc.dram_tensor("v", (HPC, 128, NKB * (D + 1)), BF16, kind="ExternalInput").ap()
    mask = nc.dram_tensor("mask", (128, 128), F32, kind="ExternalInput").ap()
    # Schraudolph bias planes (3 x 1024 cols, layouts relative to each
    # chunk's data start): plane0 = tri@0 then B16F (single-diag chunks);
    # plane1 = tri@0,tri@640 ([9,12] bin); plane2 = tri@0,tri@512 ([10,13]).
    # tri = B16F - 2^20 where s > q_local (saturates to bf16 -0.0).
    mbig = nc.dram_tensor("mbig", (128, 3 * 1024), F32, kind="ExternalInput").ap()
    # out [h, p, c*64 + d] = O[h, 128*c + p, d]
    o = nc.dram_tensor("o", (HPC, 128, NKB * D), F32, kind="ExternalOutput").ap()

    with tile.TileContext(nc) as tc:
        with (
            tc.tile_pool(name="constp", bufs=1) as constp,
            tc.tile_pool(name="qkp", bufs=3) as qkp,
            tc.tile_pool(name="ptp", bufs=PTP_BUFS) as ptp,
            tc.tile_pool(name="osbp", bufs=3) as osbp,
            tc.tile_pool(name="spp", bufs=3, space="PSUM") as spp,
            tc.tile_pool(name="opp", bufs=2, space="PSUM") as opp,
        ):
            mask_sb = constp.tile([128, 128], F32)
            mbig_sb = constp.tile([128, 3 * 1024], F32)

            def body():
                _emit_body(
                    nc, tc, qt, kt, v, o, mask_sb, mbig_sb, qkp, ptp, osbp,
                    spp, opp, mask, mbig,
                )

            if reps == 1:
                body()
            else:
                with tc.For_i(0, reps, 1):
                    body()

    nc.compile()
    _cached[key] = nc
    return nc


def _chunk_table(last_head=False):
    """Per-head chunk list in interleaved ORDER.

    Each chunk: dict(g, items, w, eng, diag) with items =
    [(kb, loc, c0, c1)]: kb's piece covers q columns [c0, c1) at tile-local
    offset loc. diag = True iff items[0] starts at its own diagonal.
    """
    per_pass = {}
    for g, (p0, p1) in enumerate(PASSES):
        lst = []
        for entry in PASS_CHUNKS[g]:
            grp, eng = entry[0], entry[1]
            offsets = entry[2] if len(entry) > 2 else None
            items, off = [], 0
            for j, kb in enumerate(grp):
                qs = 128 * kb
                c0, c1 = max(qs, p0), p1
                w = c1 - c0
                assert w > 0
                if offsets is not None:
                    off = offsets[j]
                items.append((kb, off, c0, c1))
                off += w
            assert off <= 1024
            first_off = items[0][1]
            mplane = 0
            if offsets is not None:
                mplane = MBIN_PLANES[tuple(grp)]
            diag = items[0][2] == 128 * items[0][0]
            # tile-local placement: shifting a single-item chunk by 128 turns
            # a 512+128 bank split into 384+256 (avoids the sub-256 fp32r
            # penalty); multi-item groups stay packed from 0.
            if len(items) == 1 and off % 512 == 128 and off > 128:
                items = [(kb, loc + 128, c0, c1) for kb, loc, c0, c1 in items]
            lst.append(
                {"g": g, "items": items, "eng": eng, "diag": diag,
                 "lo": items[0][1], "w": off - first_off,
                 "mplane": mplane}
            )
        per_pass[g] = lst
    assert sorted(ORDER) == sorted(
        (g, i) for g in per_pass for i in range(len(per_pass[g]))
    )
    chunks = [per_pass[g][i] for g, i in ORDER]
    # The PSUM O-bank must hold a SINGLE accumulation group: interleaved
    # per-region start/stop groups within one bank corrupt regions that stop
    # while others continue (verified on hardware). Each pass's bank is
    # opened by one pre-zero matmul; bank_last[g] marks the only stop.
    bank_last = {}
    for ci, ch in enumerate(chunks):
        for kb, loc, c0, c1 in ch["items"]:
            for qb in range(c0 // 128, c1 // 128):
                bank_last[ch["g"]] = (ci, kb, qb)
    return chunks, bank_last


def _emit_body(
    nc, tc, qt, kt, v, o, mask_sb, mbig_sb, qkp, ptp, osbp, spp, opp, mask, mbig
):
    EXP = mybir.ActivationFunctionType.Exp
    tables = [_chunk_table(last_head=(h == HPC - 1)) for h in range(HPC)]
    npc = len(tables[0][0])  # chunks per head

    sb = {}  # h -> (qt_sb, kt_sb, v_sb)
    heads = {}  # h -> dict(o_sb, rec)
    opsum = {}  # (h, g) -> psum tile [128, 512]
    sch_tiles = {}  # (h, ci) -> sch tile

    # Warm the ACT exp table immediately (overlaps the first input DMAs).
    warm = osbp.tile([128, 1], F32, tag="warm")
    nc.vector.memset(warm[:], 0.0)
    nc.scalar.activation(warm[:], warm[:], EXP, scale=0.0)
    # zeros operand for the O-bank pre-zero matmuls
    zb = osbp.tile([128, 512], BF16, tag="zb")
    nc.vector.memset(zb[:], 0.0)

    def load(h, first_=False):
        qt_sb = qkp.tile([D, T], F32R, tag="qt", name=f"qt_sb{h}")
        kt_sb = qkp.tile([D, T], F32R, tag="kt", name=f"kt_sb{h}")
        v_sb = qkp.tile([128, NKB * (D + 1)], BF16, tag="v", name=f"v_sb{h}")
        if first_:
            # fine-grained leading pieces spread over FOUR queues (scalar +
            # sync + vector HWDGE rings, gpsimd SWDGE) so the whole first
            # head lands in ~4 us. The interleaved chunk order consumes
            # pass-B columns (q >= 896) from chunk 3 on, so the vector ring
            # carries the qt tail from the start.
            nc.scalar.dma_start(kt_sb[:, :128], kt[h, :, :128])
            nc.sync.dma_start(qt_sb[:, :512], qt[h, :, :512])
            nc.sync.dma_start(qt_sb[:, 512:896], qt[h, :, 512:896])
            nc.gpsimd.dma_start(qt_sb[:, 896:], qt[h, :, 896:])
            nc.gpsimd.dma_start(kt_sb[:, 128 : T // 2], kt[h, :, 128 : T // 2])
            nc.gpsimd.dma_start(v_sb[:, : 8 * 65], v[h, :, : 8 * 65])
        else:
            # coarse prefetch on the sync HWDGE ring (cheap triggers; the
            # latency slack is ~1.5 head-periods)
            nc.sync.dma_start(kt_sb[:], kt[h])
            nc.sync.dma_start(qt_sb[:], qt[h])
            nc.sync.dma_start(v_sb[:], v[h])
        sb[h] = (qt_sb, kt_sb, v_sb)

    def load_tail(h):
        qt_sb, kt_sb, v_sb = sb[h]
        nc.gpsimd.dma_start(kt_sb[:, T // 2 :], kt[h, :, T // 2 :])
        nc.gpsimd.dma_start(v_sb[:, 8 * 65 :], v[h, :, 8 * 65 :])

    def emit_S(h, ci):
        ch = tables[h][0][ci]
        qt_sb, kt_sb, _ = sb[h]
        sch = spp.tile([128, 1024], F32, tag="s", name=f"sch{h}_{ci}")

        for kb, loc, c0, c1 in ch["items"]:
            qs = 128 * kb
            w = c1 - c0
            p = 0
            while p < w:
                pl = min(w, ((loc + p) // 512 + 1) * 512 - loc)
                nc.tensor.matmul(
                    sch[:, loc + p : loc + pl],
                    lhsT=kt_sb[:, qs : qs + 128],
                    rhs=qt_sb[:, c0 + p : c0 + pl],
                    start=True,
                    stop=True,
                )
                p = pl
        if ch["diag"] and ch["eng"] == "A":
            # Pool/GPSIMD cannot touch PSUM (BIR verifier) -> DVE mask add.
            kb0, loc0, _, _ = ch["items"][0]
            nc.vector.tensor_add(
                sch[:, loc0 : loc0 + 128], sch[:, loc0 : loc0 + 128], mask_sb[:]
            )
        sch_tiles[(h, ci)] = sch

    def emit_exp(h, ci):
        ch = tables[h][0][ci]
        lo, w = ch["lo"], ch["w"]
        sch = sch_tiles.pop((h, ci))
        ptt = ptp.tile([128, 1024], BF16, tag="pt", name=f"ptt{h}_{ci}")
        if ch["eng"] == "V":
            if ch["diag"]:
                pb = 1024 * ch["mplane"]
                nc.vector.scalar_tensor_tensor(
                    ptt[:, lo : lo + w].bitcast(I16),
                    sch[:, lo : lo + w],
                    128.0,
                    mbig_sb[:, pb : pb + w],
                    mybir.AluOpType.mult,
                    mybir.AluOpType.add,
                )
            else:
                nc.vector.tensor_scalar(
                    ptt[:, lo : lo + w].bitcast(I16),
                    sch[:, lo : lo + w],
                    128.0,
                    B16F,
                    mybir.AluOpType.mult,
                    mybir.AluOpType.add,
                )
        else:
            nc.scalar.activation(
                ptt[:, lo : lo + w], sch[:, lo : lo + w], EXP, scale=LN2
            )
        return ptt

    def emit_O(h, ci, ptt):
        ch = tables[h][0][ci]
        g = ch["g"]
        _, _, v_sb = sb[h]
        qb0_pass = PASSES[g][0] // 128
        nqb = PASSES[g][1] // 128 - qb0_pass
        if (h, g) not in opsum:
            ob = opp.tile([128, 512], F32, tag="ob", name=f"ob{h}_{g}")
            opsum[(h, g)] = ob
            # open the bank's single accumulation group, zeroing all regions
            nc.tensor.matmul(
                ob[:, : 65 * nqb],
                lhsT=zb[:, :128],
                rhs=zb[:, : 65 * nqb],
                start=True,
                stop=False,
                skip_group_check=True,
            )
        ob = opsum[(h, g)]
        for kb, loc, c0, c1 in ch["items"]:
            for qb in range(c0 // 128, c1 // 128):
                j = qb - qb0_pass
                off = loc + 128 * qb - c0
                nc.tensor.matmul(
                    ob[:, 65 * j : 65 * j + 65],
                    lhsT=ptt[:, off : off + 128],
                    rhs=v_sb[:, 65 * kb : 65 * kb + 65],
                    start=False,
                    stop=(tables[h][1][g] == (ci, kb, qb)),
                    skip_group_check=True,
                )

    def emit_epilogue(h, g):
        if h not in heads:
            heads[h] = {
                "o": osbp.tile([128, NKB * D], F32, tag="o", name=f"o_sb{h}"),
                "rec": osbp.tile([128, NKB], F32, tag="rec", name=f"rec{h}"),
            }
        o_sb, rec = heads[h]["o"], heads[h]["rec"]
        ob = opsum.pop((h, g))
        qb0 = PASSES[g][0] // 128
        nqb = PASSES[g][1] // 128 - qb0
        sums = ob[:, : nqb * 65].rearrange("p (c d) -> p c d", d=65)[:, :, 64]
        nc.vector.reciprocal(rec[:, qb0 : qb0 + nqb], sums)
        nc.vector.tensor_mul(
            o_sb[:, 64 * qb0 : 64 * (qb0 + nqb)].rearrange(
                "p (c d) -> p c d", d=64
            ),
            ob[:, : nqb * 65].rearrange("p (c d) -> p c d", d=65)[:, :, :64],
            rec[:, qb0 : qb0 + nqb].unsqueeze(2).to_broadcast((128, nqb, 64)),
        )
        nc.sync.dma_start(
            o[h, :, 64 * qb0 : 64 * (qb0 + nqb)],
            o_sb[:, 64 * qb0 : 64 * (qb0 + nqb)],
        )

    LOOKAHEAD = 2
    n = HPC * npc
    load(0, first_=True)
    # mbig is needed by chunk 0 (DVE diagonal): scalar ring (otherwise
    # idle). mask (first ACT diagonal, chunk 1) on sync behind the leading
    # qt pieces.
    nc.scalar.dma_start(mbig_sb[:, :1024], mbig[:, :1024])
    nc.sync.dma_start(mask_sb[:], mask[:])
    nc.scalar.dma_start(mbig_sb[:, 1024:], mbig[:, 1024:])
    load(1, first_=False)
    for i in range(min(LOOKAHEAD, n)):
        emit_S(i // npc, i % npc)
    pass_count = {g: len(PASS_CHUNKS[g]) for g in PASS_CHUNKS}
    deferred = {}  # emit-at global idx -> [(h, g) epilogues]
    done_cnt = {}
    for i in range(n):
        h, ci = i // npc, i % npc
        ch = tables[h][0][ci]
        ptt = emit_exp(h, ci)
        if i == 1:
            load_tail(0)
        if i + LOOKAHEAD < n:
            j = i + LOOKAHEAD
            emit_S(j // npc, j % npc)
        emit_O(h, ci, ptt)
        for hg in deferred.pop(i, []):
            emit_epilogue(*hg)
        g = ch["g"]
        done_cnt[(h, g)] = done_cnt.get((h, g), 0) + 1
        if done_cnt[(h, g)] == pass_count[g]:
            deferred.setdefault(min(i + DEFER, n - 1), []).append((h, g))
            if g == 0 and h + 2 < HPC:
                load(h + 2)
    for i in sorted(deferred):
        for hg in deferred[i]:
            emit_epilogue(*hg)
    deferred.clear()


def _prep_in_maps(Q, K, V):
    Q = np.asarray(Q, dtype=np.float32).reshape(B * NH, T, D)
    K = np.asarray(K, dtype=np.float32).reshape(B * NH, T, D)
    V = np.asarray(V, dtype=np.float32).reshape(B * NH, T, D)

    mask = np.where(
        np.arange(128)[:, None] <= np.arange(128)[None, :], 0.0, NEG
    ).astype(np.float32)
    tri = np.where(
        np.arange(128)[:, None] <= np.arange(128)[None, :], B16F, B16F - MASK_DROP
    ).astype(np.float32)
    mbig = np.full((128, 3 * 1024), B16F, dtype=np.float32)
    mbig[:, 0:128] = tri  # plane0: single leading diagonal
    mbig[:, 1024:1152] = tri  # plane1: [9,12] bin, diags at relative 0 / 640
    mbig[:, 1024 + 640 : 1024 + 768] = tri
    mbig[:, 2048:2176] = tri  # plane2: [10,13] bin, diags at relative 0 / 512
    mbig[:, 2048 + 512 : 2048 + 640] = tri
    qscale = np.float32(LOG2E / np.sqrt(D))

    in_maps = []
    for c in range(NCORES):
        hs = slice(HPC * c, HPC * (c + 1))
        qt = Q[hs].transpose(0, 2, 1) * qscale  # [hpc, 64, T], log2-domain
        kt = K[hs].transpose(0, 2, 1)
        va = np.concatenate(
            [V[hs], np.ones((HPC, T, 1), dtype=np.float32)], axis=-1
        )  # [hpc, T, 65]
        va = (
            va.reshape(HPC, NKB, 128, D + 1)
            .transpose(0, 2, 1, 3)
            .reshape(HPC, 128, NKB * (D + 1))
            .astype(ml_dtypes.bfloat16)
        )
        in_maps.append(
            {
                "qt": np.ascontiguousarray(qt.astype(np.float32)),
                "kt": np.ascontiguousarray(kt),
                "v": np.ascontiguousarray(va),
                "mask": mask,
                "mbig": mbig,
            }
        )
    return in_maps


def _gather(results):
    out = np.empty((B * NH, T, D), dtype=np.float32)
    for c in range(NCORES):
        oc = results[c]["o"]  # [HPC, 128, NKB*D]
        for s in range(HPC):
            out[HPC * c + s] = (
                oc[s].reshape(128, NKB, D).transpose(1, 0, 2).reshape(T, D)
            )
    return out.reshape(B, NH, T, D)


def _run(in_maps, **kwargs):
    nc = _build()
    return run_bass_kernel_spmd(nc, in_maps, core_ids=list(range(NCORES)), **kwargs)


def kernel(Q, K, V):
    in_maps = _prep_in_maps(Q, K, V)
    res = _run(in_maps)
    return _gather(res.results)
